# revision 18
# baseline (speedup 1.0000x reference)
"""Round-5 kernel (best: 57574 ns): per-group 512 KB loads/stores, alternating
ACT/DVE half-strip copies, PE warm-up, split first/last stores."""

import numpy as np
import ml_dtypes

BATCH = 16384
NUM_BRANCHES = 64
IN_FEATURES = 32
OUT_FEATURES = 32
D = NUM_BRANCHES * IN_FEATURES  # 2048

NUM_CORES = 8
SHARD = BATCH // NUM_CORES  # 2048 rows per core
P = 128
GROUPS = D // P  # 16
BRANCH_PER_GROUP = P // IN_FEATURES  # 4

CHUNK_N = 512
HALF = SHARD // 2  # 1024
WARMUP_MM = 12

USE_BF16 = True

_NC_CACHE = {}


def _np_io_dtype():
    return ml_dtypes.bfloat16 if USE_BF16 else np.float32


def _build_bass(use_bf16=USE_BF16):
    import concourse.mybir as mybir
    from concourse import bacc
    from concourse.tile import TileContext

    f32 = mybir.dt.float32
    fio = mybir.dt.bfloat16 if use_bf16 else f32
    shard = SHARD

    nc = bacc.Bacc("TRN2", target_bir_lowering=False, debug=False)
    xt = nc.dram_tensor("xt", [GROUPS, P, shard], fio, kind="ExternalInput")
    wbd = nc.dram_tensor("wbd", [P, D], fio, kind="ExternalInput")
    biasp = nc.dram_tensor("biasp", [P, GROUPS], f32, kind="ExternalInput")
    outp = nc.dram_tensor("outp", [GROUPS, P, shard], fio, kind="ExternalOutput")

    with TileContext(nc) as tc:
        with (
            tc.tile_pool(name="wpool", bufs=1) as wpool,
            tc.tile_pool(name="xpool", bufs=6) as xpool,
            tc.tile_pool(name="opool", bufs=4) as opool,
            tc.tile_pool(name="pspool", bufs=4, space="PSUM") as pspool,
        ):
            # bias/W ride the otherwise-empty ACT ring, in parallel with
            # the first input strips on SP: the single-queue load/store
            # stream starts ~1.6 us earlier
            b_sb = wpool.tile([P, GROUPS], f32, tag="b")
            nc.scalar.dma_start(out=b_sb[:], in_=biasp[:])
            w_sb = wpool.tile([P, D], fio, tag="w")
            nc.scalar.dma_start(out=w_sb[:], in_=wbd[:])

            junk = wpool.tile([P, CHUNK_N], fio, tag="junk")
            nc.vector.memset(junk[:], 0.0)
            psj = pspool.tile([P, HALF], f32, tag="ps", name="psj")
            for _ in range(WARMUP_MM):
                nc.tensor.matmul(
                    psj[:, :CHUNK_N], junk[:, :P], junk[:], start=True, stop=True
                )

            copy_idx = 0

            def psum_to_sbuf(dst, ps, g):
                nonlocal copy_idx
                if copy_idx % 2 == 0:
                    nc.scalar.activation(
                        dst,
                        ps,
                        mybir.ActivationFunctionType.Identity,
                        bias=b_sb[:, g : g + 1],
                    )
                else:
                    nc.vector.tensor_tensor(
                        dst,
                        ps,
                        b_sb[:, g : g + 1].to_broadcast((P, HALF)),
                        mybir.AluOpType.add,
                    )
                copy_idx += 1

            # single-queue DMA: every load and store is issued from SP in
            # program order, so the fabric services one FIFO stream of
            # alternating 512 KB transfers — no load/store arbitration, no
            # store backlog. Loads run 6 strips ahead of the store stream.
            def issue_load(g, split=False):
                xt_t = xpool.tile([P, shard], fio, tag="xt", name=f"xt{g}")
                if split:
                    # halves: the first matmuls start ~0.7 us earlier
                    for h in range(2):
                        nc.sync.dma_start(
                            out=xt_t[:, h * HALF : (h + 1) * HALF],
                            in_=xt[:][g][:, h * HALF : (h + 1) * HALF],
                        )
                else:
                    nc.sync.dma_start(out=xt_t[:], in_=xt[:][g])
                return xt_t

            xtiles = [issue_load(g, split=(g == 0)) for g in range(6)]

            for g in range(GROUPS):
                xt_t = xtiles[g]
                o_t = opool.tile([P, shard], fio, tag="o")
                for h in range(2):
                    ps = pspool.tile([P, HALF], f32, tag="ps")
                    for ci in range(2):
                        c0 = h * HALF + ci * CHUNK_N
                        nc.tensor.matmul(
                            ps[:, ci * CHUNK_N : (ci + 1) * CHUNK_N],
                            w_sb[:, g * P : (g + 1) * P],
                            xt_t[:, c0 : c0 + CHUNK_N],
                            start=True,
                            stop=True,
                        )
                    dst = o_t[:, h * HALF : (h + 1) * HALF]
                    psum_to_sbuf(dst, ps[:], g)
                for _ in range(2):
                    nc.tensor.ldweights(weights=junk[:, :P])
                if g < 2 or g == GROUPS - 1:
                    for h in range(2):
                        nc.sync.dma_start(
                            out=outp[:][g][:, h * HALF : (h + 1) * HALF],
                            in_=o_t[:, h * HALF : (h + 1) * HALF],
                        )
                else:
                    nc.sync.dma_start(out=outp[:][g], in_=o_t[:])
                if g + 6 < GROUPS:
                    xtiles.append(issue_load(g + 6))
    nc.compile()
    return nc


def _get_nc(use_bf16=USE_BF16):
    key = (use_bf16,)
    if key not in _NC_CACHE:
        _NC_CACHE[key] = _build_bass(use_bf16)
    return _NC_CACHE[key]


def _pack_wbd(W):
    W = np.asarray(W, np.float32)
    wbd = np.zeros((P, D), np.float32)
    for g in range(GROUPS):
        for j in range(BRANCH_PER_GROUP):
            b = g * BRANCH_PER_GROUP + j
            r0 = j * IN_FEATURES
            c0 = g * P + j * OUT_FEATURES
            wbd[r0 : r0 + IN_FEATURES, c0 : c0 + OUT_FEATURES] = W[b]
    return wbd.astype(_np_io_dtype())


def _pack_xt(shard):
    n = shard.shape[0]
    return np.ascontiguousarray(shard.T).astype(_np_io_dtype()).reshape(GROUPS, P, n)


def _pack_bias(b):
    return np.ascontiguousarray(np.asarray(b, np.float32).reshape(GROUPS, P).T)


def _unpack_out(outp):
    n = outp.shape[-1]
    return outp.astype(np.float32).reshape(D, n).T


def kernel(x, W, b):
    from concourse.bass_utils import run_bass_kernel_spmd

    x = np.asarray(x, np.float32)
    wbd = _pack_wbd(W)
    biasp = _pack_bias(b)

    nc = _get_nc()
    in_maps = []
    for i in range(NUM_CORES):
        shard = x[i * SHARD : (i + 1) * SHARD]
        in_maps.append({"xt": _pack_xt(shard), "biasp": biasp, "wbd": wbd})

    res = run_bass_kernel_spmd(nc, in_maps, core_ids=list(range(NUM_CORES)))
    return np.ascontiguousarray(
        np.concatenate([_unpack_out(r["outp"]) for r in res.results], axis=0)
    )
